# revision 1
# baseline (speedup 1.0000x reference)
"""Trainium2 Bass kernel for a segmented tensor-product contraction.

Computation (per batch row z, channel u, segments of width U=128):
  out[z, so, u] += c_p * x0[i0[z], s0_p, u] * prod_k x1[z, sk_p, u]
for 256 paths of degree 1..3 over S=16 segments.

Strategy:
  - Data-parallel over z across 8 NeuronCores (512 rows each).
  - On-chip layout: [u (partitions) x z (free dim)] per segment; every
    elementwise op is a [128, 512] instruction.
  - x0 row gather: host builds one-hot(i0) per core; TensorEngine computes
    x0gT[s] = x0[:, s]^T @ onehot (gather + transpose for free).
  - Factorization (globally optimized per so-group): suffix products
    sg(s0,s) = x0g[s0]*x1[s] and pairs pr(a,b) = x1[a]*x1[b]; each path is
    one tensor_tensor plus a coefficient scale on ScalarE (or a fused
    scalar_tensor_tensor on VectorE for a fraction of paths).
  - Product builds are packed into merged multi-segment instructions
    (sg runs share one instr via a stride-0 broadcast of x0g[s0]; pair
    runs along constant delta read contiguous x1 spans).
  - Output accumulation on TensorEngine: identity-matmul each path term
    into a per-segment PSUM bank (exact f32 adds). 16 output segments ->
    two groups of 8 banks; the so-partition is optimized to minimize
    duplicated product builds.
"""

import os
from collections import defaultdict

import numpy as np

U = 128
S = 16
NELEM = 64
Z = 4096
NCORES = 8
ZS = Z // NCORES  # 512 rows per core

LAST_EXEC_NS = None
LAST_RESULTS = None

F32 = "float32"


def _parse_paths(idxs, coeffs):
    paths = []  # (degree, x1segs_sorted, s0, so, coeff)
    for idx, cf in zip(idxs, coeffs):
        d = idx.shape[1] - 2
        for r, c in zip(idx, cf):
            r = [int(v) for v in r]
            paths.append((d, tuple(sorted(r[:d])), r[d], r[d + 1], float(c)))
    return paths


def _options(p):
    """Candidate (products, form) decompositions for a path.

    Each option: (frozenset of product keys, form)
    form = (in0_ref, in1_ref) with refs ('x1',s) ('x0g',s) ('sg',(s0,s))
    ('pair',(a,b)); d1 form = (('sg',(s0,s)), None).
    Product keys: ('sg',(s0,s)), ('pair',(a,b)).
    """
    d, segs, s0, so, c = p
    if d == 1:
        k = ("sg", (s0, segs[0]))
        return [(frozenset([k]), (k, None))]
    if d == 2:
        a, b = segs
        opts = [
            (frozenset([("sg", (s0, b))]), (("x1", a), ("sg", (s0, b)))),
            (frozenset([("sg", (s0, a))]), (("x1", b), ("sg", (s0, a)))),
            (frozenset([("pair", (a, b))]), (("pair", (a, b)), ("x0g", s0))),
        ]
        return opts
    a, b, cc = segs
    return [
        (
            frozenset([("pair", (a, b)), ("sg", (s0, cc))]),
            (("pair", (a, b)), ("sg", (s0, cc))),
        ),
        (
            frozenset([("pair", (a, cc)), ("sg", (s0, b))]),
            (("pair", (a, cc)), ("sg", (s0, b))),
        ),
        (
            frozenset([("pair", (b, cc)), ("sg", (s0, a))]),
            (("pair", (b, cc)), ("sg", (s0, a))),
        ),
    ]


def _optimize_group(gpaths, n_sweeps=4):
    """Choose per-path decomposition minimizing total unique products."""
    choices = [0] * len(gpaths)
    opts = [_options(p) for p in gpaths]
    for _ in range(n_sweeps):
        counts = defaultdict(int)
        for i, p in enumerate(gpaths):
            for k in opts[i][choices[i]][0]:
                counts[k] += 1
        changed = False
        for i, p in enumerate(gpaths):
            best, best_cost = choices[i], None
            for j, (prods, _) in enumerate(opts[i]):
                # marginal cost: products not used by anyone else
                cost = 0.0
                for k in prods:
                    others = counts[k] - (1 if k in opts[i][choices[i]][0] else 0)
                    cost += 1.0 / (1 + others)
                if best_cost is None or cost < best_cost - 1e-9:
                    best, best_cost = j, cost
            if best != choices[i]:
                # update counts incrementally
                for k in opts[i][choices[i]][0]:
                    counts[k] -= 1
                for k in opts[i][best][0]:
                    counts[k] += 1
                choices[i] = best
                changed = True
        if not changed:
            break
    products = set()
    forms = []
    for i, p in enumerate(gpaths):
        prods, form = opts[i][choices[i]]
        products |= prods
        forms.append(form)
    return products, forms


def _group_cost(paths, sos_a):
    """Estimate total builds for a candidate so-partition."""
    total = 0
    for sos in (sos_a, [s for s in range(S) if s not in sos_a]):
        gp = [p for p in paths if p[3] in sos]
        prods, _ = _optimize_group(gp, n_sweeps=4)
        total += len(prods)
    return total


def _optimize_partition(paths):
    """Two-stage exhaustive search of the 8/8 so-partition (C(16,8)/2 =
    6435 candidates): cheap 1-sweep proxy scan, then exact re-scoring of
    the best candidates."""
    from itertools import combinations

    def cost(sos_a, sweeps):
        total = 0
        for sos in (sos_a, [s for s in range(S) if s not in sos_a]):
            gp = [p for p in paths if p[3] in sos]
            prods, _ = _optimize_group(gp, n_sweeps=sweeps)
            total += len(prods)
        return total

    cands = [c for c in combinations(range(S), 8) if 0 in c]
    scored = sorted(cands, key=lambda c: cost(list(c), 1))[:30]
    best = min(scored, key=lambda c: cost(list(c), 4))
    cur = list(best)
    other = [s for s in range(S) if s not in cur]
    return cur, other


def _plan_merges(products):
    """Pack product builds into merged instructions.

    Returns (slot_of, builds) where slot_of maps product key -> slot index
    and builds is a list of ('sg_run', s0, s_lo, n, slot_lo) or
    ('pair_run', delta, a_lo, n, slot_lo).
    """
    slot_of = {}
    builds = []
    next_slot = 0
    sgs = defaultdict(list)  # s0 -> sorted s list
    prs = defaultdict(list)  # delta -> sorted a list
    for k in products:
        if k[0] == "sg":
            sgs[k[1][0]].append(k[1][1])
        else:
            a, b = k[1]
            prs[b - a].append(a)
    squares = sorted(prs.pop(0, []))
    # pair runs first: they depend only on x1t (no gather chain), so the
    # DVE can start on them while the x0 gather pipeline fills
    for delta in sorted(prs):
        aa = sorted(prs[delta])
        run = [aa[0]]
        for a in aa[1:] + [None]:
            if a is not None and a == run[-1] + 1:
                run.append(a)
            else:
                kind = "pair_run"
                builds.append((kind, delta, run[0], len(run), next_slot))
                for i, ra in enumerate(run):
                    slot_of[("pair", (ra, ra + delta))] = next_slot + i
                next_slot += len(run)
                if a is not None:
                    run = [a]
    for s0 in sorted(sgs):
        ss = sorted(sgs[s0])
        run = [ss[0]]
        for s in ss[1:] + [None]:
            if s is not None and s == run[-1] + 1:
                run.append(s)
            else:
                builds.append(("sg_run", s0, run[0], len(run), next_slot))
                for i, rs in enumerate(run):
                    slot_of[("sg", (s0, rs))] = next_slot + i
                next_slot += len(run)
                if s is not None:
                    run = [s]
    return slot_of, builds, next_slot, squares


def _build_plan(idxs, coeffs):
    """Full schedule. Returns (groups, all_sq).

    Joint factorization over ALL paths; products used by both so-groups
    are built once into a shared slot region and stay resident across
    both PSUM phases. Group-unique products overlay one reuse region.
    """
    paths = _parse_paths(idxs, coeffs)
    products, forms = _optimize_group(paths, n_sweeps=6)
    part_a = list(range(8))
    part_b = list(range(8, 16))

    all_sq = sorted(
        set(k[1][0] for k in products if k[0] == "pair" and k[1][0] == k[1][1])
    )
    sq_keys = set(("pair", (s, s)) for s in all_sq)

    # classify products by which groups use them
    use_a, use_b = set(), set()
    for p, form in zip(paths, forms):
        tgt = use_a if p[3] in part_a else use_b
        for r in form:
            if r and r[0] in ("sg", "pair") and r not in sq_keys:
                tgt.add(r)
    shared = use_a & use_b
    uniq = {0: use_a - shared, 1: use_b - shared}

    slot_shared, builds_shared, ns, _ = _plan_merges(shared)
    slot_a, builds_a, na, _ = _plan_merges(uniq[0])
    slot_b, builds_b, nb, _ = _plan_merges(uniq[1])
    base2 = ns
    n_main = ns + max(na, nb)
    sq_slot = {s: n_main + i for i, s in enumerate(all_sq)}
    n_slots = n_main + len(all_sq)

    def shift(builds, slot, delta):
        bs = [(b[0], b[1], b[2], b[3], b[4] + delta) for b in builds]
        sl = {k: v + delta for k, v in slot.items()}
        return bs, sl

    builds_a, slot_a = shift(builds_a, slot_a, base2)
    builds_b, slot_b = shift(builds_b, slot_b, base2)

    groups = []
    for gi, (sos, gbuilds, gslot) in enumerate(
        (
            (part_a, builds_shared + builds_a, {**slot_shared, **slot_a}),
            (part_b, builds_b, {**slot_shared, **slot_b}),
        )
    ):
        slot_of = dict(gslot)
        for s in all_sq:
            slot_of[("pair", (s, s))] = sq_slot[s]
        gidx = [i for i, p in enumerate(paths) if p[3] in sos]
        order = sorted(
            gidx,
            key=lambda i: (
                paths[i][0] != 1,
                max(
                    (
                        slot_of[r]
                        for r in forms[i]
                        if r and r[0] in ("sg", "pair")
                    ),
                    default=-1,
                ),
            ),
        )
        path_ops = [
            (paths[i][0], forms[i][0], forms[i][1], paths[i][4], paths[i][3])
            for i in order
        ]
        groups.append(
            dict(
                sos=sos,
                builds=gbuilds,
                slot_of=slot_of,
                n_slots=n_slots,
                path_ops=path_ops,
            )
        )
    return groups, all_sq


SLAB = 32  # coefficient-diagonal matrices per DMA slab


def _build_bass(groups, dtype_name, act_frac, warmup, pool_frac=0.0, all_sq=()):
    import concourse.bacc as bacc
    import concourse.mybir as mybir
    from concourse.tile import TileContext

    dt = mybir.dt.float32 if dtype_name == F32 else mybir.dt.bfloat16
    MULT = mybir.AluOpType.mult

    nc = bacc.Bacc("TRN2", debug=False)

    n_paths_total = sum(len(g["path_ops"]) for g in groups)
    n_slabs = (n_paths_total + SLAB - 1) // SLAB

    x1t_d = nc.dram_tensor("x1t", [S * U, ZS], dt, kind="ExternalInput")
    x0_d = nc.dram_tensor("x0w", [NELEM, S * U], dt, kind="ExternalInput")
    oh_d = nc.dram_tensor("oh", [NELEM, ZS], dt, kind="ExternalInput")
    cd_d = nc.dram_tensor("cdiag", [n_slabs * SLAB * U, U], dt, kind="ExternalInput")
    out_d = nc.dram_tensor("outt", [S * U, ZS], dt, kind="ExternalOutput")
    junk_d = nc.dram_tensor("junk", [U, ZS], mybir.dt.float32)

    max_slots = max(g["n_slots"] for g in groups)
    coeff_order = []  # flat list of coefficients in emission order

    with TileContext(nc) as tc:
        with tc.tile_pool(name="persist", bufs=1) as persist, tc.tile_pool(
            name="tmp", bufs=8
        ) as tmp_pool, tc.tile_pool(name="slab", bufs=2) as slab_pool:
            x1t = persist.tile([U, S * ZS], dt, tag="x1t")
            x0g = persist.tile([U, S * ZS], dt, tag="x0g")
            out_sb = persist.tile([U, S * ZS], dt, tag="out")
            prod = persist.tile([U, max_slots * ZS], dt, tag="prod")
            x0_sb = persist.tile([NELEM, S * U], dt, tag="x0w")
            oh_sb = persist.tile([NELEM, ZS], dt, tag="oh")

            def seg(t, s):
                return t[:, s * ZS : (s + 1) * ZS]

            def span(t, lo, n):
                return t[:, lo * ZS : (lo + n) * ZS]

            nc.sync.dma_start(out=oh_sb[:], in_=oh_d[:])
            nc.sync.dma_start(out=x0_sb[:], in_=x0_d[:])
            # x1t segments in order of first use by the build schedule
            seg_order = []

            def _want(s):
                if s not in seg_order:
                    seg_order.append(s)

            for s in all_sq:
                _want(s)
            for g in groups:
                for b in g["builds"]:
                    kind, key, lo, n, _ = b
                    if kind == "pair_run":
                        for i in range(n):
                            _want(lo + i)
                            _want(lo + i + key)
                    else:
                        for i in range(n):
                            _want(lo + i)
            for s in range(S):
                _want(s)
            for s in seg_order:
                nc.sync.dma_start(out=seg(x1t, s), in_=x1t_d[s * U : (s + 1) * U, :])

            # global square products on ScalarE (before any ACT Copy use,
            # emitted as consecutive runs to avoid table-set thrashing)
            if all_sq:
                max_g = groups[0]["n_slots"] - len(all_sq)
                run = [all_sq[0]]
                ri = 0
                for s in list(all_sq[1:]) + [None]:
                    if s is not None and s == run[-1] + 1:
                        run.append(s)
                    else:
                        nc.scalar.activation(
                            span(prod, max_g + ri, len(run)),
                            span(x1t, run[0], len(run)),
                            mybir.ActivationFunctionType.Square,
                        )
                        ri += len(run)
                        if s is not None:
                            run = [s]

            # PE warmup burst + gather matmuls
            with tc.tile_pool(name="gpsum", bufs=4, space="PSUM") as gpsum:
                if warmup > 0:
                    wt = gpsum.tile([U, ZS], mybir.dt.float32, tag="warm", bufs=1)
                    for i in range(warmup):
                        nc.tensor.matmul(
                            wt[:],
                            x0_sb[:, 0:U],
                            oh_sb[:],
                            start=(i == 0),
                            stop=(i == warmup - 1),
                        )
                    ws = tmp_pool.tile([U, ZS], mybir.dt.float32, tag="warms")
                    nc.scalar.copy(out=ws[:], in_=wt[:])
                    nc.sync.dma_start(out=junk_d[:], in_=ws[:])
                for s in range(S):
                    pt = gpsum.tile([U, ZS], mybir.dt.float32, tag="gps")
                    nc.tensor.matmul(
                        pt[:],
                        x0_sb[:, s * U : (s + 1) * U],
                        oh_sb[:],
                        start=True,
                        stop=True,
                    )
                    nc.scalar.copy(out=seg(x0g, s), in_=pt[:])

            slab_state = {"idx": -1, "tile": None}
            for g in groups:
                sos, builds, slot_of, path_ops = (
                    g["sos"],
                    g["builds"],
                    g["slot_of"],
                    g["path_ops"],
                )
                # interleave: emit builds, releasing paths when ready
                ready_after = defaultdict(list)  # build idx -> path indices
                path_needs = []
                for i, (d, r1, r2, c, so) in enumerate(path_ops):
                    needs = set()
                    for r in (r1, r2):
                        if r and r[0] in ("sg", "pair"):
                            needs.add(slot_of[r])
                    path_needs.append(needs)
                slot_done_at = {}
                for bi, b in enumerate(builds):
                    for i in range(b[3]):
                        slot_done_at[b[4] + i] = bi
                for i, needs in enumerate(path_needs):
                    bi = max(
                        (slot_done_at.get(s, -1) for s in needs), default=-1
                    )
                    ready_after[bi].append(i)

                # emission order determines PE program order: derive
                # first/last per so from it for the start/stop flags
                emit_order = list(ready_after[-1])
                for bi in range(len(builds)):
                    emit_order.extend(ready_after[bi])
                first_for_so = {}
                last_for_so = {}
                for i in emit_order:
                    so = path_ops[i][4]
                    if so not in first_for_so:
                        first_for_so[so] = i
                    last_for_so[so] = i

                acc = {}
                with tc.tile_pool(
                    name=f"acc{sos[0]}", bufs=8, space="PSUM"
                ) as acc_pool:
                    for so in sos:
                        if so in first_for_so:
                            acc[so] = acc_pool.tile(
                                [U, ZS],
                                mybir.dt.float32,
                                tag=f"acc{sos.index(so)}",
                                name=f"acc_{so}",
                                bufs=1,
                            )

                    def pref(r):
                        kind, key = r
                        if kind == "x1":
                            return seg(x1t, key)
                        if kind == "x0g":
                            return seg(x0g, key)
                        sl = slot_of[r]
                        return seg(prod, sl)

                    def emit_path(i):
                        d, r1, r2, c, so = path_ops[i]
                        gi = len(coeff_order)
                        coeff_order.append(c)
                        sj, sk = gi // SLAB, gi % SLAB
                        if slab_state["idx"] != sj:
                            slab_state["idx"] = sj
                            st = slab_pool.tile(
                                [U, SLAB * U], dt, tag="slab", name=f"slab{sj}"
                            )
                            slab_state["tile"] = st
                            nc.sync.dma_start(
                                out=st[:].rearrange("p (d c) -> p d c", d=SLAB),
                                in_=cd_d[sj * SLAB * U : (sj + 1) * SLAB * U, :]
                                .rearrange("(d p) c -> p d c", p=U),
                            )
                        st = slab_state["tile"]
                        if d == 1:
                            rhs = pref(r1)
                        else:
                            t1 = tmp_pool.tile([U, ZS], dt, tag="tmp", name=f"t1{i}")
                            nc.vector.tensor_tensor(
                                out=t1[:], in0=pref(r1), in1=pref(r2), op=MULT
                            )
                            rhs = t1[:]
                        nc.tensor.matmul(
                            acc[so][:],
                            st[:, sk * U : (sk + 1) * U],
                            rhs,
                            start=(i == first_for_so[so]),
                            stop=(i == last_for_so[so]),
                        )

                    for i in ready_after[-1]:
                        emit_path(i)
                    for bi, b in enumerate(builds):
                        kind = b[0]
                        if kind == "sq_run":
                            _, delta, a_lo, n, slot_lo = b
                            nc.scalar.activation(
                                span(prod, slot_lo, n),
                                span(x1t, a_lo, n),
                                mybir.ActivationFunctionType.Square,
                            )
                        elif kind == "sg_run":
                            _, s0, s_lo, n, slot_lo = b
                            in0 = (
                                seg(x0g, s0)
                                .rearrange("p (o z) -> p o z", o=1)
                                .broadcast_to([U, n, ZS])
                            )
                            in1 = span(x1t, s_lo, n).rearrange(
                                "p (r z) -> p r z", r=n
                            )
                            out = span(prod, slot_lo, n).rearrange(
                                "p (r z) -> p r z", r=n
                            )
                            nc.vector.tensor_tensor(
                                out=out, in0=in0, in1=in1, op=MULT
                            )
                        else:
                            _, delta, a_lo, n, slot_lo = b
                            in0 = span(x1t, a_lo, n).rearrange(
                                "p (r z) -> p r z", r=n
                            )
                            in1 = span(x1t, a_lo + delta, n).rearrange(
                                "p (r z) -> p r z", r=n
                            )
                            out = span(prod, slot_lo, n).rearrange(
                                "p (r z) -> p r z", r=n
                            )
                            nc.vector.tensor_tensor(
                                out=out, in0=in0, in1=in1, op=MULT
                            )
                        for i in ready_after[bi]:
                            emit_path(i)

                    for so in sos:
                        if so in acc:
                            nc.scalar.copy(out=seg(out_sb, so), in_=acc[so][:])
                        else:
                            nc.vector.memset(seg(out_sb, so), 0.0)

            for s in range(S):
                nc.sync.dma_start(out=out_d[s * U : (s + 1) * U, :], in_=seg(out_sb, s))

    nc.compile()
    return nc, coeff_order


def kernel(x0, x1, coeff1, coeff2, coeff3, i0, idx1, idx2, idx3):
    global LAST_EXEC_NS, LAST_RESULTS
    from concourse.bass_utils import run_bass_kernel_spmd

    x0 = np.asarray(x0, dtype=np.float32)
    x1 = np.asarray(x1, dtype=np.float32)
    i0 = np.asarray(i0).astype(np.int64)
    idxs = [np.asarray(a) for a in (idx1, idx2, idx3)]
    coeffs = [np.asarray(c, dtype=np.float32) for c in (coeff1, coeff2, coeff3)]

    dtype_name = os.environ.get("KERNEL_DTYPE", "bfloat16")
    act_frac = float(os.environ.get("KERNEL_ACT_FRAC", "0.55"))
    pool_frac = float(os.environ.get("KERNEL_POOL_FRAC", "0.3"))
    warmup = int(os.environ.get("KERNEL_WARMUP", "0"))
    npdt = np.float32
    if dtype_name != F32:
        import ml_dtypes

        npdt = ml_dtypes.bfloat16

    groups, all_sq = _build_plan(idxs, coeffs)
    nc, coeff_order = _build_bass(groups, dtype_name, act_frac, warmup, pool_frac, all_sq)
    n_slabs = (len(coeff_order) + SLAB - 1) // SLAB
    cdiag = np.zeros((n_slabs * SLAB * U, U), dtype=npdt)
    for gi, c in enumerate(coeff_order):
        blk = cdiag[gi * U : (gi + 1) * U, :]
        np.fill_diagonal(blk, np.asarray(c, dtype=npdt))

    in_maps = []
    eye = np.arange(NELEM)
    x0c = x0.astype(npdt)
    for c in range(NCORES):
        zl, zh = c * ZS, (c + 1) * ZS
        shard = x1[zl:zh]
        x1t = np.ascontiguousarray(
            shard.reshape(ZS, S, U).transpose(1, 2, 0).reshape(S * U, ZS)
        ).astype(npdt)
        oh = (i0[zl:zh][None, :] == eye[:, None]).astype(npdt)
        in_maps.append({"x1t": x1t, "x0w": x0c, "oh": oh, "cdiag": cdiag})

    trace = os.environ.get("BASS_TRACE", "") not in ("", "0")
    trace_cores = None
    tc_env = os.environ.get("KERNEL_TRACE_CORES", "")
    if tc_env:
        trace_cores = [int(x) for x in tc_env.split(",")]
    res = run_bass_kernel_spmd(
        nc, in_maps, core_ids=list(range(NCORES)), trace=trace,
        trace_cores=trace_cores,
    )
    LAST_EXEC_NS = res.exec_time_ns
    LAST_RESULTS = res

    out = np.empty((Z, S * U), dtype=np.float32)
    for c in range(NCORES):
        outt = np.asarray(res.results[c]["outt"], dtype=np.float32)
        out[c * ZS : (c + 1) * ZS] = (
            outt.reshape(S, U, ZS).transpose(2, 0, 1).reshape(ZS, S * U)
        )
    return out



# revision 8
# speedup vs baseline: 1.0454x; 1.0454x over previous
"""Trainium2 Bass kernel for a segmented tensor-product contraction.

Computation (per batch row z, channel u, segments of width U=128):
  out[z, so, u] += c_p * x0[i0[z], s0_p, u] * prod_k x1[z, sk_p, u]
for 256 paths of degree 1..3 over S=16 segments.

Strategy (v2):
  - Data-parallel over z across 8 NeuronCores (512 rows each).
  - One big SBUF "arena" of [128 x 512z] bf16 slots: x1t segs | x0g segs |
    shared aux | phase-overlaid aux | squares | pins | term pool | out
    staging.  Every DVE op is a strided multi-slot TENSOR_TENSOR over
    arena slots; any two tiles merge (arbitrary slot strides via AP
    middle-dim striding), longer runs merge on arithmetic progressions.
    This amortizes the ~150 ns per-instruction DVE overhead measured on
    hardware against the ~267 ns/tile streaming cost.
  - x0 row gather via one-hot matmuls on PE (doubles as PE warmup).
  - Joint CSE over all 256 paths into shared sg(s0,s)/pair(a,b) products;
    squares x1[s]^2 go to the otherwise-idle ACT engine.
  - Coefficient scaling + accumulation on PE: diag(c) matmul per path use
    into per-so PSUM banks (2 phases x 8 banks), exact f32 adds.
  - PE keep-warm: zero-diag filler matmuls during DVE-only stretches keep
    the HAM clock gate at 2.4 GHz.
"""

import os
from collections import Counter, defaultdict

import numpy as np

U = 128
S = 16
NELEM = 64
Z = 4096
NCORES = 8
ZS = Z // NCORES  # 512 rows per core

LAST_EXEC_NS = None
LAST_RESULTS = None

F32 = "float32"
SLAB = 16  # coefficient-diagonal matrices per DMA slab
MAX_ARENA = 196  # max arena slots (KB/partition budget)


# --------------------------------------------------------------------------
# planning
# --------------------------------------------------------------------------

def _parse_paths(idxs, coeffs):
    paths = []  # (degree, x1segs_sorted, s0, so, coeff)
    for idx, cf in zip(idxs, coeffs):
        d = idx.shape[1] - 2
        for r, c in zip(idx, cf):
            r = [int(v) for v in r]
            paths.append((d, tuple(sorted(r[:d])), r[d], r[d + 1], float(c)))
    return paths


def _pairkey(a, b):
    if a == b:
        return ("sq", a)
    return ("pair", (min(a, b), max(a, b)))


def _mono_options(segs, s0):
    """Split options for a distinct d>=2 monomial: (aux_keys, (refA, refB)).
    refs: ('x1',s) ('x0g',s0) ('sg',(s0,s)) ('pair',(a,b)) ('sq',a)."""
    d = len(segs)
    if d == 2:
        a, b = segs
        if a == b:
            return [
                ((("sq", a),), (("sq", a), ("x0g", s0))),
                ((("sg", (s0, a)),), (("sg", (s0, a)), ("x1", a))),
            ]
        return [
            ((("sg", (s0, a)),), (("sg", (s0, a)), ("x1", b))),
            ((("sg", (s0, b)),), (("sg", (s0, b)), ("x1", a))),
            ((("pair", (a, b)),), (("pair", (a, b)), ("x0g", s0))),
        ]
    a, b, c = segs
    opts = []
    seen = set()
    for x, (y, z) in ((a, (b, c)), (b, (a, c)), (c, (a, b))):
        pk = _pairkey(y, z)
        key = (("sg", (s0, x)), pk)
        if key in seen:
            continue
        seen.add(key)
        opts.append((key, (pk, ("sg", (s0, x)))))
    return opts


def _choose_splits(monos, d1sgs, n_sweeps=8, seed=0):
    import random

    rng = random.Random(seed)
    keys = [m for m in monos if len(m[0]) >= 2]
    opts = {m: _mono_options(*m) for m in keys}
    choice = {m: 0 for m in keys}
    counts = defaultdict(int)

    def kcost(k):
        if k[0] == "sq":
            return 0.02
        if k in d1sgs:
            return 0.0
        return 1.0

    for m in keys:
        for k in opts[m][choice[m]][0]:
            counts[k] += 1
    for _ in range(n_sweeps):
        changed = False
        order = keys[:]
        rng.shuffle(order)
        for m in order:
            cur = choice[m]
            best, best_c = cur, None
            for ci, (aux, _) in enumerate(opts[m]):
                cost = 0.0
                for k in aux:
                    others = counts[k] - (1 if k in opts[m][cur][0] else 0)
                    cost += kcost(k) / (1 + others)
                if best_c is None or cost < best_c - 1e-9:
                    best, best_c = ci, cost
            if best != cur:
                for k in opts[m][cur][0]:
                    counts[k] -= 1
                for k in opts[m][best][0]:
                    counts[k] += 1
                choice[m] = best
                changed = True
        if not changed:
            break
    forms = {m: opts[m][choice[m]][1] for m in keys}
    aux = set()
    for m in keys:
        for k in opts[m][choice[m]][0]:
            aux.add(k)
    return forms, aux


MAXD = 63  # max |slot stride| — TENSOR3D step_elem is 16-bit (x512 elems)


def _merge_ops(items):
    """Group (A_slot, B_slot) ops into strided runs.

    Returns list of (member_idx_list, a0, da, b0, db); strides in slots,
    0 = broadcast, |stride| <= MAXD.  Members are ordered; dests assigned
    contiguously by the caller in member order."""
    remaining = set(range(len(items)))
    runs = []
    banned = set()
    while True:
        cnt = Counter()
        for i in remaining:
            a, b = items[i]
            cnt[a] += 1
            cnt[b] += 1
        cand = [(n, v) for v, n in cnt.items() if n >= 2 and v not in banned]
        if not cand:
            break
        _, v = max(cand)
        grp = [i for i in remaining if v in items[i]]
        if len(grp) < 2:
            banned.add(v)
            continue

        def other(i):
            a, b = items[i]
            return b if a == v else a

        others = sorted(grp, key=other)
        chains = []
        cur = [others[0]]
        for i in others[1:]:
            d = other(i) - other(cur[-1])
            if len(cur) == 1:
                if abs(d) <= MAXD:
                    cur.append(i)
                else:
                    chains.append(cur)
                    cur = [i]
            elif d == other(cur[1]) - other(cur[0]):
                cur.append(i)
            else:
                chains.append(cur)
                cur = [i]
        chains.append(cur)
        got = False
        for ch in chains:
            if len(ch) >= 2:
                got = True
                da = other(ch[1]) - other(ch[0])
                runs.append((ch, other(ch[0]), da, v, 0))
                for i in ch:
                    remaining.discard(i)
        if not got:
            banned.add(v)

    def pair_run(i1, i2):
        (a1, b1), (a2, b2) = items[i1], items[i2]
        if abs(a2 - a1) <= MAXD and abs(b2 - b1) <= MAXD:
            return ([i1, i2], a1, a2 - a1, b1, b2 - b1)
        if abs(b2 - a1) <= MAXD and abs(a2 - b1) <= MAXD:
            return ([i1, i2], a1, b2 - a1, b1, a2 - b1)
        return None

    left = sorted(remaining, key=lambda i: items[i])
    used = set()
    for idx, i1 in enumerate(left):
        if i1 in used:
            continue
        for i2 in left[idx + 1:]:
            if i2 in used:
                continue
            r = pair_run(i1, i2)
            if r is not None:
                runs.append(r)
                used.add(i1)
                used.add(i2)
                break
        else:
            runs.append(([i1], items[i1][0], 1, items[i1][1], 1))
            used.add(i1)
    return runs


def _plan(idxs, coeffs, npool):
    paths = _parse_paths(idxs, coeffs)
    monos = defaultdict(list)  # (segs, s0) -> [(so, c)]
    for d, segs, s0, so, c in paths:
        monos[(segs, s0)].append((so, c))
    monos = dict(monos)
    d1sgs = set(("sg", (m[1], m[0][0])) for m in monos if len(m[0]) == 1)
    forms, aux = _choose_splits(monos, d1sgs)
    for k in d1sgs:
        aux.add(k)

    def phase_of_so(so):
        return 0 if so < 8 else 1

    # phase usage of each aux key and monomial
    aux_phases = defaultdict(set)
    mono_phase = {}
    for m, uses in monos.items():
        ph = set(phase_of_so(so) for so, _ in uses)
        mono_phase[m] = ph
        if len(m[0]) == 1:
            for p in ph:
                aux_phases[("sg", (m[1], m[0][0]))].add(p)
        else:
            for r in forms[m]:
                if r[0] in ("sg", "pair", "sq"):
                    for p in ph:
                        aux_phases[r].add(p)

    sqs = sorted(k for k in aux if k[0] == "sq")
    shared = sorted(k for k in aux if k[0] != "sq" and len(aux_phases[k]) != 1)
    only = {ph: sorted(k for k in aux
                       if k[0] != "sq" and aux_phases[k] == {ph})
            for ph in (0, 1)}

    # ---- slot layout -------------------------------------------------
    # [x1 0..15][x0g 16..31][shared][overlay max(A,B)][sq][pins][pool][stage]
    slot_of = {}  # x1 / x0g / shared / sq keys
    slot_ph = {0: {}, 1: {}}  # phase-only aux keys
    for s in range(S):
        slot_of[("x1", s)] = s
        slot_of[("x0g", s)] = S + s

    def op_slots(k):
        if k[0] == "pair":
            a, b = k[1]
            return (slot_of[("x1", a)], slot_of[("x1", b)])
        s0, s = k[1]
        return (slot_of[("x0g", s0)], slot_of[("x1", s)])

    def merged_builds(keys, start, assign):
        """Merge build ops for aux `keys`, assign dest slots from `start` in
        run-member order via assign(key, slot).  Returns (runs, end)."""
        keys = list(keys)
        mruns = _merge_ops([op_slots(k) for k in keys])
        b = start
        runs = []
        for members, a0, da, b0, db in mruns:
            for i in members:
                assign(keys[i], b)
                b += 1
            runs.append((b - len(members), len(members), a0, da, b0, db))
        return runs, b

    cur = 2 * S
    runs_sh, cur = merged_builds(
        shared, cur, lambda k, s: slot_of.__setitem__(k, s))
    ov_base = cur
    runs_ao, end_a = merged_builds(
        only[0], ov_base, lambda k, s: slot_ph[0].__setitem__(k, s))
    runs_bo, end_b = merged_builds(
        only[1], ov_base, lambda k, s: slot_ph[1].__setitem__(k, s))
    cur = max(end_a, end_b)
    sq_base = cur
    for k in sqs:
        slot_of[k] = cur
        cur += 1
    pins = sorted(m for m in monos if len(m[0]) >= 2 and len(mono_phase[m]) > 1)
    pin_slot = {}
    for m in pins:
        pin_slot[m] = cur
        cur += 1
    # shrink pool if over budget
    npool = max(8, min(npool, MAX_ARENA - 4 - cur))
    pool_base = cur
    stage_base = pool_base + npool
    n_slots = stage_base + 4

    def res(ph):
        def f(k):
            if k in slot_of:
                return slot_of[k]
            return slot_ph[ph][k]
        return f

    # phase-A build runs: shared + A-only, x0g-independent (pair) runs first
    def needs_x0g(run):
        d0, k, a0, da, b0, db = run
        for i in range(k):
            for v in (a0 + i * da, b0 + i * db):
                if S <= v < 2 * S:
                    return 1
        return 0

    runsA = runs_sh + runs_ao
    runsA.sort(key=lambda r: (needs_x0g(r), r[0]))
    runsB = runs_bo

    # square runs (ACT): dests contiguous from sq_base, srcs = x1 slot = seg
    sq_runs = []
    i = 0
    while i < len(sqs):
        j = i + 1
        if j < len(sqs):
            ds = sqs[j][1] - sqs[i][1]
            while j < len(sqs) and sqs[j][1] - sqs[j - 1][1] == ds:
                j += 1
        sq_runs.append((sq_base + i, j - i, sqs[i][1],
                        (sqs[i + 1][1] - sqs[i][1]) if j - i > 1 else 1))
        i = j

    # ---- terms -------------------------------------------------------
    terms = {0: [], 1: []}  # (mono, A_slot, B_slot)
    d1_mms = {0: [], 1: []}  # (sg_slot, so, c)
    for m, uses in sorted(monos.items()):
        segs, s0 = m
        if len(segs) == 1:
            sgk = ("sg", (s0, segs[0]))
            for so, c in uses:
                ph = phase_of_so(so)
                d1_mms[ph].append((res(ph)(sgk), so, c))
            continue
        ph = min(mono_phase[m])
        rA, rB = forms[m]
        terms[ph].append((m, res(ph)(rA), res(ph)(rB)))

    term_runs = {ph: _merge_ops([(t[1], t[2]) for t in terms[ph]])
                 for ph in (0, 1)}

    # readiness of phase-A term runs / d1 mms vs. runsA emission order
    slot_ready = {}
    for ri, (d0, k, *_r) in enumerate(runsA):
        for i in range(k):
            slot_ready[d0 + i] = ri
    nA = len(runsA)

    def rdy(slots):
        return max([slot_ready.get(s, -1) for s in slots] + [-1])

    readyA = [min(nA - 1, rdy([s for mi in mem
                               for s in terms[0][mi][1:3]]))
              for (mem, *_r) in term_runs[0]]
    d1_readyA = [min(nA - 1, rdy([sl])) for sl, _, _ in d1_mms[0]]

    return dict(
        monos=monos, forms=forms, paths=paths, mono_phase=mono_phase,
        runsA=runsA, runsB=runsB, sq_runs=sq_runs,
        terms=terms, term_runs=term_runs, readyA=readyA,
        d1_mms=d1_mms, d1_readyA=d1_readyA,
        pins=pins, pin_slot=pin_slot,
        pool_base=pool_base, npool=npool, stage_base=stage_base,
        n_slots=n_slots, phase_of_so=phase_of_so,
    )


def _schedule(plan):
    """Interleaved emission schedule + full PE matmul order."""
    termsA, termsB = plan["terms"][0], plan["terms"][1]
    runsA_t, runsB_t = plan["term_runs"][0], plan["term_runs"][1]
    monos = plan["monos"]
    phase_of_so = plan["phase_of_so"]
    pin_slot = plan["pin_slot"]
    npool = plan["npool"]
    pool_base = plan["pool_base"]
    nbuildA = len(plan["runsA"])

    runs_after = defaultdict(list)
    for ti, r in enumerate(plan["readyA"]):
        runs_after[r].append(ti)
    d1_after = defaultdict(list)
    for di, r in enumerate(plan["d1_readyA"]):
        d1_after[r].append(di)

    pool_next = [0]

    def alloc_pool(k):
        if pool_next[0] + k > npool:
            pool_next[0] = 0
        lo = pool_next[0]
        pool_next[0] += k
        return pool_base + lo

    term_dest = {}

    def place(ph, terms_ph, run):
        members = run[0]
        dest = alloc_pool(len(members))
        for j, mi in enumerate(members):
            m = terms_ph[mi][0]
            term_dest[(ph, mi)] = (pin_slot[m] if m in pin_slot
                                   else dest + j)

    schedule = []
    for bi in range(-1, nbuildA):
        if bi >= 0:
            schedule.append(("buildA", bi))
        for di in d1_after.get(bi, []):
            schedule.append(("d1A", di))
        for ti in runs_after.get(bi, []):
            place(0, termsA, runsA_t[ti])
            schedule.append(("termA", ti))
            for mi in runsA_t[ti][0]:
                schedule.append(("mmA", mi))
    schedule.append(("drainA",))
    schedule.append(("pinB",))
    for bi in range(len(plan["runsB"])):
        schedule.append(("buildB", bi))
    for di in range(len(plan["d1_mms"][1])):
        schedule.append(("d1B", di))
    for ti, run in enumerate(runsB_t):
        place(1, termsB, run)
        schedule.append(("termB", ti))
        for mi in run[0]:
            schedule.append(("mmB", mi))
    schedule.append(("drainB",))

    pin_b_mms = []
    for mi, t in enumerate(termsA):
        m = t[0]
        if m in pin_slot:
            for so, c in monos[m]:
                if phase_of_so(so) == 1:
                    pin_b_mms.append((so, c, pin_slot[m]))

    full_order = []
    for ev in schedule:
        kind = ev[0]
        if kind == "d1A":
            full_order.append(plan["d1_mms"][0][ev[1]])
        elif kind == "d1B":
            full_order.append(plan["d1_mms"][1][ev[1]])
        elif kind == "mmA":
            m = termsA[ev[1]][0]
            dest = term_dest[(0, ev[1])]
            for so, c in monos[m]:
                if phase_of_so(so) == 0:
                    full_order.append((dest, so, c))
        elif kind == "mmB":
            m = termsB[ev[1]][0]
            dest = term_dest[(1, ev[1])]
            for so, c in monos[m]:
                if phase_of_so(so) == 1:
                    full_order.append((dest, so, c))
        elif kind == "pinB":
            for so, c, sl in pin_b_mms:
                full_order.append((sl, so, c))
    # normalize d1 entries (sl, so, c) ordering
    norm = []
    for e in full_order:
        sl, so, c = e
        norm.append((sl, so, c))
    return schedule, term_dest, pin_b_mms, norm


# --------------------------------------------------------------------------
# bass emission
# --------------------------------------------------------------------------

def _build_bass(plan, dtype_name, filler, pool_frac):
    import concourse.bacc as bacc
    import concourse.mybir as mybir
    from concourse.tile import TileContext

    dt = mybir.dt.float32 if dtype_name == F32 else mybir.dt.bfloat16
    MULT = mybir.AluOpType.mult

    nc = bacc.Bacc("TRN2", debug=False)

    schedule, term_dest, pin_b_mms, full_order = _schedule(plan)
    first_mm, last_mm = {}, {}
    for i, (sl, so, c) in enumerate(full_order):
        if so not in first_mm:
            first_mm[so] = i
        last_mm[so] = i
    n_mms = len(full_order)
    n_slabs = (n_mms + SLAB - 1) // SLAB

    n_slots = plan["n_slots"]
    termsA, termsB = plan["terms"][0], plan["terms"][1]
    runsA_t, runsB_t = plan["term_runs"][0], plan["term_runs"][1]
    monos = plan["monos"]
    phase_of_so = plan["phase_of_so"]
    pin_slot = plan["pin_slot"]

    x1t_d = nc.dram_tensor("x1t", [S * U, ZS], dt, kind="ExternalInput")
    x0_d = nc.dram_tensor("x0w", [NELEM, S * U], dt, kind="ExternalInput")
    oh_d = nc.dram_tensor("oh", [NELEM, ZS], dt, kind="ExternalInput")
    cd_d = nc.dram_tensor("cdiag", [n_slabs * SLAB * U, U], dt,
                          kind="ExternalInput")
    out_d = nc.dram_tensor("outt", [S * U, ZS], dt, kind="ExternalOutput")

    coeff_order = []

    sosA = sorted(set(so for _, so, _ in full_order if so < 8))
    sosB = sorted(set(so for _, so, _ in full_order if so >= 8))
    filler_soA = max(sosA, key=lambda so: last_mm[so]) if sosA else None

    with TileContext(nc) as tc:
        with tc.tile_pool(name="persist", bufs=1) as persist, tc.tile_pool(
            name="slab", bufs=2
        ) as slab_pool, tc.tile_pool(name="small", bufs=1) as small:
            arena = persist.tile([U, n_slots * ZS], dt, tag="arena")
            x0_sb = small.tile([NELEM, S * U], dt, tag="x0w")
            oh_sb = small.tile([NELEM, ZS], dt, tag="oh")
            zdiag = small.tile([U, U], dt, tag="zdiag")

            ar = arena[:].rearrange("p (r z) -> p r z", r=n_slots)

            def span(lo, step, k):
                if k == 1:
                    return ar[:, lo, :]
                if step == 0:
                    return ar[:, lo:lo + 1, :].broadcast_to([U, k, ZS])
                return ar[:, lo::step, :][:, 0:k, :]

            def tile2d(slot):
                return ar[:, slot, :]

            nc.gpsimd.memset(zdiag[:], 0.0)
            nc.sync.dma_start(out=oh_sb[:], in_=oh_d[:])
            nc.sync.dma_start(out=x0_sb[:], in_=x0_d[:])
            for ci in range(4):
                q = nc.sync if ci % 2 == 0 else nc.scalar
                lo = ci * 4
                q.dma_start(
                    out=arena[:, lo * ZS:(lo + 4) * ZS]
                    .rearrange("p (k z) -> p k z", k=4),
                    in_=x1t_d[lo * U:(lo + 4) * U, :]
                    .rearrange("(k p) z -> p k z", p=U),
                )

            # squares on ACT (only need x1t)
            for d0, k, s0_, ds in plan["sq_runs"]:
                nc.scalar.activation(
                    span(d0, 1, k),
                    span(s0_, ds, k),
                    mybir.ActivationFunctionType.Square,
                )

            # gather matmuls (PE warmup) + x0g copies to arena
            with tc.tile_pool(name="gpsum", bufs=4, space="PSUM") as gpsum:
                for s in range(S):
                    pt = gpsum.tile([U, ZS], mybir.dt.float32, tag="gps")
                    nc.tensor.matmul(
                        pt[:], x0_sb[:, s * U:(s + 1) * U], oh_sb[:],
                        start=True, stop=True,
                    )
                    nc.scalar.copy(out=tile2d(S + s), in_=pt[:])

            slab_state = {"tiles": {}, "issued": -1}

            def issue_slab(sj):
                if sj > slab_state["issued"] and sj < n_slabs:
                    st = slab_pool.tile(
                        [U, SLAB * U], dt, tag="slab", name=f"slab{sj}"
                    )
                    slab_state["tiles"][sj] = st
                    slab_state["issued"] = sj
                    nc.scalar.dma_start(
                        out=st[:].rearrange("p (d c) -> p d c", d=SLAB),
                        in_=cd_d[sj * SLAB * U:(sj + 1) * SLAB * U, :]
                        .rearrange("(d p) c -> p d c", p=U),
                    )

            def mm(rhs_slot, so, c, acc):
                gi = len(coeff_order)
                coeff_order.append(c)
                sj, sk = gi // SLAB, gi % SLAB
                issue_slab(sj)
                issue_slab(sj + 1)
                st = slab_state["tiles"][sj]
                if sk == SLAB - 1:
                    slab_state["tiles"].pop(sj - 2, None)
                nc.tensor.matmul(
                    acc[so][:], st[:, sk * U:(sk + 1) * U], tile2d(rhs_slot),
                    start=(gi == first_mm[so]), stop=(gi == last_mm[so]),
                )

            def emit_build(run, engine):
                d0, k, a0, da, b0, db = run
                engine.tensor_tensor(
                    out=span(d0, 1, k), in0=span(a0, da, k),
                    in1=span(b0, db, k), op=MULT,
                )

            def emit_term_run(run, terms_ph, ph):
                members, a0, da, b0, db = run
                pinned = any(terms_ph[mi][0] in pin_slot for mi in members)
                k = len(members)
                if not pinned and k > 1:
                    d0 = term_dest[(ph, members[0])]
                    nc.vector.tensor_tensor(
                        out=span(d0, 1, k), in0=span(a0, da, k),
                        in1=span(b0, db, k), op=MULT,
                    )
                else:
                    for mi in members:
                        _, As, Bs = terms_ph[mi]
                        nc.vector.tensor_tensor(
                            out=span(term_dest[(ph, mi)], 1, 1),
                            in0=span(As, 1, 1), in1=span(Bs, 1, 1), op=MULT,
                        )

            stage_next = [0]

            def drain(acc, sos):
                for so in sos:
                    sl = plan["stage_base"] + (stage_next[0] % 4)
                    stage_next[0] += 1
                    nc.scalar.copy(out=tile2d(sl), in_=acc[so][:])
                    nc.sync.dma_start(
                        out=out_d[so * U:(so + 1) * U, :], in_=tile2d(sl)
                    )

            n_pool_builds = int(round(len(plan["runsA"]) * pool_frac))
            pool_build_set = set()
            if n_pool_builds:
                # offload evenly-spaced build runs (skip the first few)
                idxs_ = list(range(2, len(plan["runsA"])))
                step = max(1, len(idxs_) // n_pool_builds)
                pool_build_set = set(idxs_[::step][:n_pool_builds])

            with tc.tile_pool(name="accA", bufs=8, space="PSUM") as accpA:
                accA = {so: accpA.tile(
                    [U, ZS], mybir.dt.float32, tag=f"accA{so % 8}",
                    name=f"acc_{so}", bufs=1) for so in sosA}
                for ev in schedule:
                    kind = ev[0]
                    if kind == "buildA":
                        eng = (nc.gpsimd if ev[1] in pool_build_set
                               else nc.vector)
                        emit_build(plan["runsA"][ev[1]], eng)
                        for _ in range(filler):
                            nc.tensor.matmul(
                                accA[filler_soA][:], zdiag[:], tile2d(0),
                                start=False, stop=False,
                            )
                    elif kind == "d1A":
                        sl, so, c = plan["d1_mms"][0][ev[1]]
                        mm(sl, so, c, accA)
                    elif kind == "termA":
                        emit_term_run(runsA_t[ev[1]], termsA, 0)
                    elif kind == "mmA":
                        mi = ev[1]
                        m = termsA[mi][0]
                        dest = term_dest[(0, mi)]
                        for so, c in monos[m]:
                            if phase_of_so(so) == 0:
                                mm(dest, so, c, accA)
                    elif kind == "drainA":
                        drain(accA, sosA)
                        break
            with tc.tile_pool(name="accB", bufs=8, space="PSUM") as accpB:
                accB = {so: accpB.tile(
                    [U, ZS], mybir.dt.float32, tag=f"accB{so % 8}",
                    name=f"acc_{so}", bufs=1) for so in sosB}
                after = False
                for ev in schedule:
                    kind = ev[0]
                    if kind == "drainA":
                        after = True
                        continue
                    if not after:
                        continue
                    if kind == "pinB":
                        for so, c, sl in pin_b_mms:
                            mm(sl, so, c, accB)
                    elif kind == "buildB":
                        emit_build(plan["runsB"][ev[1]], nc.vector)
                    elif kind == "d1B":
                        sl, so, c = plan["d1_mms"][1][ev[1]]
                        mm(sl, so, c, accB)
                    elif kind == "termB":
                        emit_term_run(runsB_t[ev[1]], termsB, 1)
                    elif kind == "mmB":
                        mi = ev[1]
                        m = termsB[mi][0]
                        dest = term_dest[(1, mi)]
                        for so, c in monos[m]:
                            if phase_of_so(so) == 1:
                                mm(dest, so, c, accB)
                    elif kind == "drainB":
                        drain(accB, sosB)

    nc.compile()
    assert len(coeff_order) == n_mms, (len(coeff_order), n_mms)
    return nc, coeff_order


# --------------------------------------------------------------------------
# host wrapper
# --------------------------------------------------------------------------

def kernel(x0, x1, coeff1, coeff2, coeff3, i0, idx1, idx2, idx3):
    global LAST_EXEC_NS, LAST_RESULTS
    from concourse.bass_utils import run_bass_kernel_spmd

    x0 = np.asarray(x0, dtype=np.float32)
    x1 = np.asarray(x1, dtype=np.float32)
    i0 = np.asarray(i0).astype(np.int64)
    idxs = [np.asarray(a) for a in (idx1, idx2, idx3)]
    coeffs = [np.asarray(c, dtype=np.float32) for c in (coeff1, coeff2, coeff3)]

    dtype_name = os.environ.get("KERNEL_DTYPE", "bfloat16")
    npool = int(os.environ.get("KERNEL_NPOOL", "20"))
    filler = int(os.environ.get("KERNEL_FILLER", "1"))
    pool_frac = float(os.environ.get("KERNEL_POOL_FRAC", "0.0"))
    npdt = np.float32
    if dtype_name != F32:
        import ml_dtypes

        npdt = ml_dtypes.bfloat16

    plan = _plan(idxs, coeffs, npool)
    nc, coeff_order = _build_bass(plan, dtype_name, filler, pool_frac)
    n_slabs = (len(coeff_order) + SLAB - 1) // SLAB
    cdiag = np.zeros((n_slabs * SLAB * U, U), dtype=npdt)
    for gi, c in enumerate(coeff_order):
        blk = cdiag[gi * U:(gi + 1) * U, :]
        np.fill_diagonal(blk, np.asarray(c, dtype=npdt))

    in_maps = []
    eye = np.arange(NELEM)
    x0c = x0.astype(npdt)
    for c in range(NCORES):
        zl, zh = c * ZS, (c + 1) * ZS
        shard = x1[zl:zh]
        x1t = np.ascontiguousarray(
            shard.reshape(ZS, S, U).transpose(1, 2, 0).reshape(S * U, ZS)
        ).astype(npdt)
        oh = (i0[zl:zh][None, :] == eye[:, None]).astype(npdt)
        in_maps.append({"x1t": x1t, "x0w": x0c, "oh": oh, "cdiag": cdiag})

    trace = os.environ.get("BASS_TRACE", "") not in ("", "0")
    trace_cores = None
    tc_env = os.environ.get("KERNEL_TRACE_CORES", "")
    if tc_env:
        trace_cores = [int(x) for x in tc_env.split(",")]
    res = run_bass_kernel_spmd(
        nc, in_maps, core_ids=list(range(NCORES)), trace=trace,
        trace_cores=trace_cores,
    )
    LAST_EXEC_NS = res.exec_time_ns
    LAST_RESULTS = res

    have_so = set(so for _, _, _, so, _ in plan["paths"])
    out = np.empty((Z, S * U), dtype=np.float32)
    for c in range(NCORES):
        outt = np.asarray(res.results[c]["outt"], dtype=np.float32)
        o = outt.reshape(S, U, ZS).transpose(2, 0, 1).copy()
        for so in range(S):
            if so not in have_so:
                o[:, so, :] = 0.0
        out[c * ZS:(c + 1) * ZS] = o.reshape(ZS, S * U)
    return out
